# revision 8
# baseline (speedup 1.0000x reference)
"""SAGAN-style attention block on 8 Trainium2 NeuronCores.

Math (per batch b):
  theta = W_theta @ x + b_theta            [8, 4096]
  phi   = maxpool2(W_phi @ x + b_phi)      [8, 1024]
  g     = maxpool2(W_g   @ x + b_g)        [32, 1024]
  E[m,n] = exp(S^T[m,n]), S^T[m,n] = sum_c phi[c,m] theta[c,n]
  O_aug = [g; ones] @ E                    [33, 4096]  (row 32 = softmax denom)
  o     = O_aug[0:32] / O_aug[32]
  out   = x + gamma*(W_o @ o + b_o)

Sharding: batch dim (16) split across 8 cores, 2 batches/core; weights
replicated.  No max-subtraction in softmax: |S| <= ~3 so exp is safe and
mathematically identical.  Matmuls are bf16; accumulation fp32.  The
residual uses the bf16 copy of x (~2e-3 rel err against the 2e-2 budget,
saves 2MB/core of DMA).

Schedule: the attention phase is a flat software pipeline over 32
(chunk, pair-group) stages -- at stage s the PE runs S^T(s+1) and O(s-1)
while the scalar engine runs exp(s), so exp (the largest single-engine
load) never waits.  The next batch's projection chunks are interleaved
into the current batch's attention stages to kill the batch-boundary
bubble.  Maxpool runs on the otherwise-idle gpsimd engine.  exp stages
alternate between the scalar engine (activation Exp) and a one-op
Schraudolph exp on the vector engine when EXP_DVE is set.
"""

import ml_dtypes
import numpy as np

import concourse.bass as bass
import concourse.mybir as mybir
import concourse.tile as tile
from concourse import bacc
from concourse.bass_utils import run_bass_kernel_spmd
from concourse.masks import make_identity

B, C, H, W = 16, 64, 64, 64
N = H * W            # 4096 pixels
M = N // 4           # 1024 pooled pixels
NCORES = 8
BPC = B // NCORES    # 2 batches per core
CT = C // 8          # 8 theta/phi channels
CG = C // 2          # 32 g channels
NC = 512             # n-chunk width
NCH = N // NC        # 8 chunks
MT = 128             # m-tile (partitions)
MTS = M // MT        # 8 m-tiles
GRP = 2              # m-tiles per exp group ([128, 1024] PSUM staging)
NGRP = MTS // GRP    # 4 pair-groups per chunk
NST = NCH * NGRP     # 32 attention stages per batch

# Which exp stages run as 1-op Schraudolph exp on DVE (by stage index mod
# EXP_DVE_MOD < EXP_DVE): 0 disables.
EXP_DVE = 0
EXP_DVE_MOD = 3
# Schraudolph constants for bf16-bits output: bits = x*A16 + B16 as int16,
# reinterpreted as bf16.  (bf16 exponent LSB = bit 7.)
SEXP_A = (1 << 7) * 1.4426950408889634
SEXP_B = 127.0 * (1 << 7) - 0.043677 * (1 << 7)

F32 = mybir.dt.float32
BF16 = mybir.dt.bfloat16
I16 = mybir.dt.int16
EXP = mybir.ActivationFunctionType.Exp
MAX = mybir.AluOpType.max
ADD = mybir.AluOpType.add
MULT = mybir.AluOpType.mult


def build_bass(loop_n=None, variant="full"):
    """loop_n: if set, wrap the computation in a hardware For_i loop
    repeating it loop_n times (benchmarking only).
    variant: "full" | "x<k>" (repeat body k times per iteration)."""
    import contextlib

    repeat = 1
    if variant.startswith("x"):
        repeat, variant = int(variant[1:]), "full"

    nc = bacc.Bacc("TRN2", target_bir_lowering=False, debug=False)

    xbf_d = nc.dram_tensor("xbf", [BPC, C, N], BF16, kind="ExternalInput").ap()
    wallt_d = nc.dram_tensor("w_all_t", [C, 104], BF16,
                             kind="ExternalInput").ap()
    ball_d = nc.dram_tensor("bias_all", [104, 1], F32,
                            kind="ExternalInput").ap()
    wot_d = nc.dram_tensor("wot_aug", [33, C], BF16, kind="ExternalInput").ap()
    out_d = nc.dram_tensor("out", [BPC, C, N], F32, kind="ExternalOutput").ap()

    with tile.TileContext(nc) as tc:
        with (
            tc.tile_pool(name="consts", bufs=1) as consts,
            tc.tile_pool(name="perbatch", bufs=2) as pb,
            tc.tile_pool(name="epool", bufs=4) as ep,
            tc.tile_pool(name="small", bufs=3) as sm,
            tc.tile_pool(name="spsum", bufs=2, space="PSUM") as s_psum,
            tc.tile_pool(name="projpsum", bufs=2, space="PSUM") as sp_proj,
            tc.tile_pool(name="otpsum", bufs=2, space="PSUM") as sp_ot,
        ):
            wallt = consts.tile([C, 104], BF16)
            nc.sync.dma_start(out=wallt, in_=wallt_d)
            ball = consts.tile([104, 1], F32)
            nc.sync.dma_start(out=ball, in_=ball_d)
            wot = consts.tile([33, C], BF16)
            nc.sync.dma_start(out=wot, in_=wot_d)
            ident = consts.tile([CG, CG], BF16)
            make_identity(nc, ident)

            env = dict(locals())
            loop_cm = (tc.For_i(0, loop_n, 1) if loop_n
                       else contextlib.nullcontext())
            with loop_cm:
                emit_all(nc, tc, env, repeat)
    nc.compile()
    return nc


class BatchCtx:
    """Per-batch tiles + emission helpers.  Constructing one emits the
    input DMA and the small memsets."""

    def __init__(self, nc, env, b):
        self.nc, self.env, self.b = nc, env, b
        pb = env["pb"]
        self.xbf = pb.tile([C, N], BF16, tag="xbf", name="xbf")
        self.proj = pb.tile([104, N], BF16, tag="proj", name="proj")
        # pooled g (rows 0:32) / phi (rows 32:40)
        self.pgp = pb.tile([40, M], BF16, tag="pgp", name="pgp")
        self.gaT = pb.tile([MT, MTS, 33], BF16, tag="gaT", name="gaT")
        self.onorm = pb.tile([33, N], BF16, tag="onorm", name="onorm")
        self.outb = pb.tile([C, N], F32, tag="outb", name="outb")
        nc.gpsimd.memset(self.gaT[:, :, 32], 1.0)
        nc.gpsimd.memset(self.onorm[32:33, :], 1.0)
        xbf_d = env["xbf_d"]
        nc.sync.dma_start(out=self.xbf[:, 0:N // 2],
                          in_=xbf_d[b][:, 0:N // 2])
        nc.sync.dma_start(out=self.xbf[:, N // 2:N],
                          in_=xbf_d[b][:, N // 2:N])
        self.ot = None

    def proj_chunk(self, j):
        nc, env = self.nc, self.env
        js = slice(j * NC, (j + 1) * NC)
        pj = env["sp_proj"].tile([104, NC], F32, tag="pj", name="pj")
        nc.tensor.matmul(pj, env["wallt"], self.xbf[:, js],
                         start=True, stop=True)
        nc.vector.tensor_scalar_add(out=self.proj[:, js], in0=pj,
                                    scalar1=env["ball"])
        # 2x2 maxpool of g/phi rows (walrus rejects strided tensor_tensor
        # on the Pool engine, so these stay on DVE)
        mjs = slice(j * 128, (j + 1) * 128)
        ch = self.proj[64:104, js].rearrange("p (w t) -> p w t", t=2)
        wm = env["sm"].tile([40, 256], BF16, tag="wm", name="wm")
        nc.vector.tensor_tensor(out=wm, in0=ch[:, :, 0], in1=ch[:, :, 1],
                                op=MAX)
        wmv = wm.rearrange("p (h t w) -> p h t w", t=2, w=W // 2)
        po = self.pgp[:, mjs].rearrange("p (h w) -> p h w", w=W // 2)
        nc.vector.tensor_tensor(out=po, in0=wmv[:, :, 0, :],
                                in1=wmv[:, :, 1, :], op=MAX)

    def transposes(self):
        nc, env = self.nc, self.env
        gt = env["sp_proj"].tile([MT, MTS * CG], BF16, tag="pj", name="gt")
        for i in range(MTS):
            nc.tensor.transpose(gt[:, i * CG:(i + 1) * CG],
                                self.pgp[0:CG, i * MT:(i + 1) * MT],
                                env["ident"])
        nc.vector.tensor_copy(
            out=self.gaT[:, :, 0:32],
            in_=gt.rearrange("p (i c) -> p i c", c=CG))

    # -- attention stages: s in [0, NST), (j, g) = divmod(s, NGRP) --

    def s_mm(self, s):
        nc, env = self.nc, self.env
        j, g = divmod(s, NGRP)
        js = slice(j * NC, (j + 1) * NC)
        st = env["s_psum"].tile([MT, GRP * NC], F32, tag="st", name="st")
        for t in range(GRP):
            i = GRP * g + t
            nc.tensor.matmul(st[:, t * NC:(t + 1) * NC],
                             self.pgp[32:40, i * MT:(i + 1) * MT],
                             self.proj[32:40, js],
                             start=True, stop=True)
        return st

    def exp(self, s, st):
        nc, env = self.nc, self.env
        et = env["ep"].tile([MT, GRP * NC], BF16, tag="et", name="et")
        if EXP_DVE and (s % EXP_DVE_MOD) < EXP_DVE:
            eti = et.bitcast(I16)
            nc.vector.tensor_scalar(out=eti, in0=st, scalar1=SEXP_A,
                                    scalar2=SEXP_B, op0=MULT, op1=ADD)
        else:
            nc.scalar.activation(out=et, in_=st, func=EXP)
        return et

    def o_mm(self, s, et):
        nc, env = self.nc, self.env
        j, g = divmod(s, NGRP)
        if g == 0:
            self.ot = env["sp_ot"].tile([33, NC], F32, tag="ot", name="ot")
        for t in range(GRP):
            i = GRP * g + t
            nc.tensor.matmul(self.ot, self.gaT[:, i, :],
                             et[:, t * NC:(t + 1) * NC],
                             start=(i == 0), stop=(i == MTS - 1))
        if g == NGRP - 1:
            return (j, self.ot)
        return None

    def tail(self, j, ot):
        nc, env = self.nc, self.env
        js = slice(j * NC, (j + 1) * NC)
        rs = env["sm"].tile([1, NC], F32, tag="rs", name="rs")
        nc.vector.reciprocal(out=rs, in_=ot[32:33, :])
        r32 = env["sm"].tile([CG, NC], F32, tag="r32", name="r32")
        nc.gpsimd.partition_broadcast(r32, rs)
        nc.vector.tensor_tensor(out=self.onorm[0:32, js], in0=ot[0:32, :],
                                in1=r32, op=MULT)
        # out = x + gamma*(W_o @ o + b_o)  (gamma/b_o folded into wot)
        ut = env["sp_proj"].tile([C, NC], F32, tag="pj", name="ut")
        nc.tensor.matmul(ut, env["wot"], self.onorm[:, js],
                         start=True, stop=True)
        nc.vector.tensor_tensor(out=self.outb[:, js], in0=ut,
                                in1=self.xbf[:, js], op=ADD)

    def store(self):
        nc, env = self.nc, self.env
        out_d = env["out_d"]
        nc.sync.dma_start(out=out_d[self.b][:, 0:N // 2],
                          in_=self.outb[:, 0:N // 2])
        nc.sync.dma_start(out=out_d[self.b][:, N // 2:N],
                          in_=self.outb[:, N // 2:N])


def run_attention(cur, nxt):
    """Flat 32-stage pipeline: at stage s emit exp(s), S^T(s+1), O(s-1);
    chunk tails one chunk late; next batch's proj chunks at g==2 stages."""
    sts = {0: cur.s_mm(0)}
    ets = {}
    pend_tail = None
    for s in range(NST):
        j, g = divmod(s, NGRP)
        ets[s] = cur.exp(s, sts.pop(s))
        if s + 1 < NST:
            sts[s + 1] = cur.s_mm(s + 1)
        if s >= 1:
            done = cur.o_mm(s - 1, ets.pop(s - 1))
            if done is not None:
                pend_tail = done
        if g == 1 and pend_tail is not None:
            cur.tail(*pend_tail)
            pend_tail = None
        if g == 2 and nxt is not None:
            nxt.proj_chunk(j)
    done = cur.o_mm(NST - 1, ets.pop(NST - 1))
    if pend_tail is not None:
        cur.tail(*pend_tail)
    if nxt is not None:
        nxt.transposes()
    cur.tail(*done)
    cur.store()


def emit_all(nc, tc, env, repeat=1):
    order = list(range(BPC)) * repeat
    cur = BatchCtx(nc, env, order[0])
    for j in range(NCH):
        cur.proj_chunk(j)
    cur.transposes()
    for i in range(len(order)):
        nxt = BatchCtx(nc, env, order[i + 1]) if i + 1 < len(order) else None
        run_attention(cur, nxt)
        cur = nxt


_NC_CACHE = None


def _get_nc():
    global _NC_CACHE
    if _NC_CACHE is None:
        _NC_CACHE = build_bass()
    return _NC_CACHE


def prep_in_maps(inputs, W_theta, b_theta, W_phi, b_phi, W_g, b_g, W_o, b_o,
                 gamma, **_unused):
    inputs = np.asarray(inputs, np.float32)
    W_all = np.zeros((104, C), np.float32)
    W_all[0:CT] = np.asarray(W_theta, np.float32)
    W_all[32:32 + CT] = np.asarray(W_theta, np.float32)
    W_all[64:64 + CG] = np.asarray(W_g, np.float32)
    W_all[96:96 + CT] = np.asarray(W_phi, np.float32)
    W_all_t = np.ascontiguousarray(W_all.T.astype(ml_dtypes.bfloat16))
    bias_all = np.zeros((104, 1), np.float32)
    bias_all[0:CT, 0] = np.asarray(b_theta, np.float32)
    bias_all[32:32 + CT, 0] = np.asarray(b_theta, np.float32)
    bias_all[64:64 + CG, 0] = np.asarray(b_g, np.float32)
    bias_all[96:96 + CT, 0] = np.asarray(b_phi, np.float32)
    g = np.float32(np.asarray(gamma, np.float32))
    wot_aug = np.ascontiguousarray(
        (np.concatenate([np.asarray(W_o, np.float32).T,
                         np.asarray(b_o, np.float32)[None, :]], axis=0)
         * g).astype(ml_dtypes.bfloat16))

    xbf = inputs.reshape(B, C, N).astype(ml_dtypes.bfloat16)
    in_maps = []
    for c in range(NCORES):
        in_maps.append({
            "xbf": np.ascontiguousarray(xbf[c * BPC:(c + 1) * BPC]),
            "w_all_t": W_all_t,
            "bias_all": bias_all,
            "wot_aug": wot_aug,
        })
    return in_maps


def kernel(**inputs):
    in_maps = prep_in_maps(**inputs)
    nc = _get_nc()
    res = run_bass_kernel_spmd(nc, in_maps, core_ids=list(range(NCORES)))
    out = np.concatenate([res.results[c]["out"] for c in range(NCORES)], axis=0)
    return out.reshape(B, C, H, W)


if __name__ == "__main__":
    rng = np.random.default_rng(0)
    ins = {
        "inputs": rng.standard_normal((B, C, H, W)).astype(np.float32),
        "W_theta": (rng.standard_normal((CT, C)) * 0.05).astype(np.float32),
        "b_theta": np.zeros(CT, np.float32),
        "W_phi": (rng.standard_normal((CT, C)) * 0.05).astype(np.float32),
        "b_phi": np.zeros(CT, np.float32),
        "W_g": (rng.standard_normal((CG, C)) * 0.05).astype(np.float32),
        "b_g": np.zeros(CG, np.float32),
        "W_o": (rng.standard_normal((C, CG)) * 0.05).astype(np.float32),
        "b_o": np.zeros(C, np.float32),
        "gamma": np.float32(0.5),
    }
    print(kernel(**ins).shape)
